# revision 10
# baseline (speedup 1.0000x reference)
"""Trainium2 Bass kernel for nn_DiffeqExactTraceMLP.

Math (B=1024, D=128, DH=64, H=512):
  h = MADE_fwd(x) + MADE_rev(x)                       # hollow conditioner
  u[b,i] = [t, x[b,i], h[b,i,:]]                      # [B, D, DH+2]
  y   = MLP(u)        (tanh, tanh, linear->scalar)    # per-dim MLP
  jac = exact JVP of MLP wrt the x slot of u

Sharding: hybrid 4 batch-shards x 2 dim-shards over 8 cores (each core:
256 batch rows x 64 output dims). MADE weights replicated; W2/host prep
is per-dim-shard. All masking / transposes / t-folding done on host
(cached across calls).

Per-core device layout (dimwise rows r = i_loc*256 + b, i-major):
  MADE L0/L1 run feature-on-partition ([512f, 256b] tiles).
  MADE L2 produces output directly in (dh, i)-on-partition layout: W2 is
  host-rearranged (w2r) so each [128, 1024] PSUM octet holds dh=0..63
  for 8 consecutive dims; ACT/DVE copies move PSUM halves straight into
  UT[0:64, ...] -- no single-partition DMA scatter (the v1 bottleneck).
  UT = [65, 16384]: partitions 0..63 = h features, partition 64 = x row.
  Dimwise MLP per row block of 1024 (16 blocks):
    L0 (col-major):  z1 = tanh(dw0x^T UT + bias0),  bias0 = d_b0+t*W0[0]
                     z1d = a - a*z1^2  (a = d_W0[1]; DVE)
    L1 (row-major):  per 128-row chunk, z1/z1d chunks are the stationary
      operands, W1 / (W1*-d_W2) stream:  p2T, p2jT = [128 rows, 512 hid]
    z2 = tanh(p2T) (ACT); sq2 = z2^2 (gpsimd; z1^2 also on gpsimd)
    y   = sum_h d_W2[h]*z2[:,h]    via DVE stt accum_out (w2rep folded)
    jac = sum_h (1-z2^2)*w2*p2d    via DVE stt accum_out (w2 in dw1j)
  y/jac accumulate into [128, 128] SBUF mats, two DMAs out per tensor.
Matmuls bf16 (fp32 PSUM); zero d_b1 / MADE b2 paths elided via flags.
Elementwise squares run on the otherwise-idle gpsimd engine; no DMA
issues sit on the ACT queue before the first relus. CoreSim device time
290.5us/core (baseline 1190us); HW unroll-slope measured the mid-series
body at ~141us vs 379us baseline (2.7x) before the final tweaks.
"""

import numpy as np
import ml_dtypes

B, D, DH, H = 1024, 128, 64, 512
NCORES = 8
NB, ND = 4, 2             # hybrid sharding: 4 batch shards x 2 dim shards
BS = B // NB              # batch rows per core = 256
DCORE = D // ND           # dims per core = 64
ROWS = BS * DCORE         # dimwise rows per core = 16384
RBLK = 1024               # dimwise row-block
NBLK = ROWS // RBLK       # 16
KIN = DH + 1              # 65 = [h0..h63, x]
NOCT = DCORE // 8         # 8 octets of 8 dims each

_BF = ml_dtypes.bfloat16


def _bf(x):
    return np.ascontiguousarray(np.asarray(x, dtype=np.float32).astype(_BF))


def _f32(x):
    return np.ascontiguousarray(np.asarray(x, dtype=np.float32))


def _chunk_col(v):
    """[512] -> [128, 4] with v[c*128 + p] at [p, c] (per-partition scalars)."""
    return np.ascontiguousarray(np.asarray(v, np.float32).reshape(4, 128).T)


def _made_masks(reverse):
    deg_in = np.arange(D)
    if reverse:
        deg_in = deg_in[::-1].copy()
    deg_h = np.arange(H) % (D - 1)
    degs = [deg_in, deg_h, deg_h]
    masks = [(d0[:, None] <= d1[None, :]) for d0, d1 in zip(degs[:-1], degs[1:])]
    out = degs[-1][:, None] < deg_in[None, :]
    masks.append(np.tile(out, (1, DH)))
    masks.append(out)  # untiled [H, D] hollow mask
    return [m.astype(np.float32) for m in masks]


def _l2_perm(d0):
    """Column permutation for w2r: new col q*512 + j*128 + p holds old
    (global) col dh*128 + i with dh = p % 64, i = d0 + 8q + 4*(p//64) + j."""
    q = np.arange(NOCT)[:, None, None]
    j = np.arange(4)[None, :, None]
    p = np.arange(128)[None, None, :]
    dh = p % 64
    i = d0 + 8 * q + 4 * (p // 64) + j
    return (dh * 128 + i).reshape(-1)  # [4096] old col index per new col


_NC_CACHE = {}
_MASK_CACHE = {}
_PREP_CACHE = {}


def _masks(p):
    if p not in _MASK_CACHE:
        _MASK_CACHE[p] = _made_masks(p == "m2")
    return _MASK_CACHE[p]


def _w2r_both(W2, mask_out):
    """Masked + permuted W2 for both dim-shards in one pass.

    Old cols [dh(64), i(128)] with i = ds*64 + 8q + 4hf + j ->
    new [ds][q*512 + j*128 + hf*64 + dh]. Returns bf16 [2][512, 4096]."""
    w = W2.reshape(H, 64, 128) * mask_out[:, None, :]        # [f, dh, i]
    w = w.reshape(H, 64, 2, NOCT, 2, 4)                       # [f, dh, ds, q, hf, j]
    w = np.ascontiguousarray(w.transpose(2, 0, 3, 5, 4, 1))   # [ds, f, q, j, hf, dh]
    w = w.reshape(2, H, DCORE * DH).astype(_BF)
    return [np.ascontiguousarray(w[0]), np.ascontiguousarray(w[1])]


def _build_nc(nz_b1=False, nz_b2=False):
    import os
    unroll = int(os.environ.get("BENCH_UNROLL", "1"))
    key = (unroll, nz_b1, nz_b2)
    if key in _NC_CACHE:
        return _NC_CACHE[key]
    import concourse.bacc as bacc
    import concourse.mybir as mybir
    from concourse.tile import TileContext

    dt = mybir.dt
    AF = mybir.ActivationFunctionType
    OP = mybir.AluOpType

    nc = bacc.Bacc(None, target_bir_lowering=False)

    def inp(name, shape, dtype):
        return nc.declare_dram_parameter(name, list(shape), dtype, isOutput=False)

    xT = inp("xT", (D, BS), dt.bfloat16)
    xflat = inp("xflat", (1, ROWS), dt.bfloat16)
    w0m = {p: inp(f"w0m_{p}", (D, H), dt.bfloat16) for p in ("m1", "m2")}
    w1m = {p: inp(f"w1m_{p}", (H, H), dt.bfloat16) for p in ("m1", "m2")}
    w2r = {p: inp(f"w2r_{p}", (H, DCORE * DH), dt.bfloat16) for p in ("m1", "m2")}
    b0c = {p: inp(f"b0c_{p}", (128, 4), dt.float32) for p in ("m1", "m2")}
    b1c = {p: inp(f"b1c_{p}", (128, 4), dt.float32) for p in ("m1", "m2")}
    b2r = inp("b2r", (1, DCORE * DH), dt.bfloat16) if nz_b2 else None
    dw0x = inp("dw0x", (KIN, H), dt.bfloat16)
    bias0c = inp("bias0c", (128, 4), dt.float32)
    ac = inp("ac", (128, 4), dt.float32)
    negac = inp("negac", (128, 4), dt.float32)
    dw1 = inp("dw1", (H, H), dt.bfloat16)
    dw1j = inp("dw1j", (H, H), dt.bfloat16)   # W1 * (-d_W2) per column
    b1row = inp("b1row", (1, H), dt.bfloat16) if nz_b1 else None
    w2rep = inp("w2rep", (128, H), dt.bfloat16)  # d_W2 row replicated

    y_out = nc.declare_dram_parameter("y_mat", [128, 128], dt.float32, isOutput=True)
    j_out = nc.declare_dram_parameter("jac_mat", [128, 128], dt.float32, isOutput=True)

    with TileContext(nc) as tc:
        _pools = []

        def _pool(**kw):
            p = tc.alloc_tile_pool(**kw)
            _pools.append(p)
            return p

        cpool = _pool(name="const", bufs=1)
        hpool = _pool(name="made_h", bufs=16)
        wspool = _pool(name="w2s", bufs=16)
        zpool = _pool(name="z", bufs=8)
        z1dpool = _pool(name="z1d", bufs=8)
        z2pool = _pool(name="z2", bufs=8)
        z2dpool = _pool(name="z2d", bufs=8)
        sqpool = _pool(name="sq", bufs=6)

        _w2dma_i = [0]

        def dma_w2(out, in_):
            # first 2 groups feed from sync/gpsimd only (scalar is busy with
            # the MADE relus); later groups also use the scalar queue
            rot = ([nc.sync, nc.gpsimd] if _w2dma_i[0] < 16
                   else [nc.sync, nc.gpsimd, nc.scalar])
            eng = rot[_w2dma_i[0] % len(rot)]
            _w2dma_i[0] += 1
            eng.dma_start(out=out, in_=in_)

        def load_const(src, shape, dtype, eng=None):
            t = cpool.tile(list(shape), dtype, tag=src.name, name=src.name)
            (eng or nc.sync).dma_start(out=t[:], in_=src[:])
            return t

        # NOTE: no DMAs on nc.scalar before the MADE relus — DMA issues
        # occupy the ACT queue and delay the first activations.
        xT_sb = load_const(xT, (D, BS), dt.bfloat16)
        w0m_sb = {p: load_const(w0m[p], (D, H), dt.bfloat16, eng=nc.gpsimd)
                  for p in ("m1", "m2")}
        b0c_sb = {p: load_const(b0c[p], (128, 4), dt.float32) for p in ("m1", "m2")}
        # W1 chunk rows k*128.. as [128, 4*512]: chunk (k,m) at [:, k*512+m*128]
        w1m_sb = {}
        for p in ("m1", "m2"):
            t = cpool.tile([128, 4 * H], dt.bfloat16, tag=f"w1m_sb_{p}", name=f"w1m_sb_{p}")
            for k in range(4):
                eng = nc.gpsimd if k % 2 == 0 else nc.sync
                eng.dma_start(
                    out=t[:, k * H:(k + 1) * H], in_=w1m[p][k * 128:(k + 1) * 128, :]
                )
            w1m_sb[p] = t
        b1c_sb = {p: load_const(b1c[p], (128, 4), dt.float32, eng=nc.sync) for p in ("m1", "m2")}
        b2r_sb = load_const(b2r, (1, DCORE * DH), dt.bfloat16) if nz_b2 else None
        dw0x_sb = cpool.tile([KIN, H], dt.bfloat16, tag="dw0x", name="dw0x")
        bias0c_sb = cpool.tile([128, 4], dt.float32, tag="bias0c", name="bias0c")
        ac_sb = cpool.tile([128, 4], dt.float32, tag="ac", name="ac")
        negac_sb = cpool.tile([128, 4], dt.float32, tag="negac", name="negac")
        w2rep_sb = cpool.tile([128, H], dt.bfloat16, tag="w2rep", name="w2rep")
        b1row_sb = cpool.tile([1, H], dt.bfloat16, tag="b1row", name="b1row") if nz_b1 else None
        dw1_sb = cpool.tile([128, 4 * H], dt.bfloat16, tag="dw1_sb", name="dw1_sb")
        dw1j_sb = cpool.tile([128, 4 * H], dt.bfloat16, tag="dw1j_sb", name="dw1j_sb")

        def load_dimwise_consts():
            # emitted after the MADE L2 DMA burst; sync/gpsimd drain w2 first
            nc.gpsimd.dma_start(out=dw0x_sb[:], in_=dw0x[:])
            nc.gpsimd.dma_start(out=bias0c_sb[:], in_=bias0c[:])
            nc.gpsimd.dma_start(out=ac_sb[:], in_=ac[:])
            nc.gpsimd.dma_start(out=negac_sb[:], in_=negac[:])
            nc.sync.dma_start(out=w2rep_sb[:], in_=w2rep[:])
            if nz_b1:
                nc.sync.dma_start(out=b1row_sb[:], in_=b1row[:])
            for k in range(4):
                nc.sync.dma_start(
                    out=dw1_sb[:, k * H:(k + 1) * H], in_=dw1[k * 128:(k + 1) * 128, :]
                )
                nc.gpsimd.dma_start(
                    out=dw1j_sb[:, k * H:(k + 1) * H], in_=dw1j[k * 128:(k + 1) * 128, :]
                )
        y_sb = cpool.tile([128, 128], dt.float32, tag="y_sb", name="y_sb")
        j_sb = cpool.tile([128, 128], dt.float32, tag="j_sb", name="j_sb")

        ones1 = cpool.tile([1, BS], dt.bfloat16, tag="ones1", name="ones1")
        nc.vector.memset(ones1[:], 1.0)

      # repeated body for benchmarking (BENCH_UNROLL>1); rep results identical

        for _rep in range(unroll):
            UT = cpool.tile([KIN, ROWS], dt.bfloat16, tag="UT", name="UT")

            def load_xrow():
                # x row (partition 64), split across queues to shorten the
                # single-partition write
                xeng = [nc.sync, nc.scalar, nc.gpsimd, nc.scalar]
                bounds = [0, 4096, 8192, 12288, ROWS]
                for s in range(4):
                    sl = slice(bounds[s], bounds[s + 1])
                    xeng[s].dma_start(out=UT[DH:DH + 1, sl], in_=xflat[0:1, sl])

            # ---------------- MADE (both orderings) ----------------
            madeps = tc.alloc_tile_pool(name="madeps", bufs=4, space="PSUM")
            # PE warmup: throwaway matmuls on the memset ones tile while the
            # first input DMAs land — ramps the PE p-state (HAM) so the real
            # MADE matmuls start at full clock
            if _rep == 0:
                wps = madeps.tile([128, BS], dt.float32, tag="mps", name="warm")
                for _w in range(14):
                    nc.tensor.matmul(
                        wps[:, 0:128], ones1[0:1, 0:128], ones1[0:1, 0:128],
                        start=True, stop=True,
                    )
            l2ps = tc.alloc_tile_pool(name="l2ps", bufs=2, space="PSUM")
            h1T = {}
            h2T = {}
            for p in ("m1", "m2"):
                for c in range(4):
                    ps = madeps.tile([128, BS], dt.float32, tag="mps", name="mps")
                    nc.tensor.matmul(
                        ps[:], w0m_sb[p][:, c * 128:(c + 1) * 128], xT_sb[:],
                        start=True, stop=True,
                    )
                    h = hpool.tile([128, BS], dt.bfloat16, tag="h", name="h")
                    nc.scalar.activation(h[:], ps[:], AF.Relu, bias=b0c_sb[p][:, c:c + 1])
                    h1T[p, c] = h
            for p in ("m1", "m2"):
                for m in range(4):
                    ps = madeps.tile([128, BS], dt.float32, tag="mps", name="mps")
                    for k in range(4):
                        nc.tensor.matmul(
                            ps[:],
                            w1m_sb[p][:, k * H + m * 128: k * H + (m + 1) * 128],
                            h1T[p, k][:],
                            start=(k == 0), stop=(k == 3),
                        )
                    h = hpool.tile([128, BS], dt.bfloat16, tag="h2", name="h2")
                    nc.scalar.activation(h[:], ps[:], AF.Relu, bias=b1c_sb[p][:, m:m + 1])
                    h2T[p, m] = h

            # MADE layer 2: per octet q (8 dims), accumulate into [128, 512]
            # PSUM with (dh, i-half) on partitions; copy halves into UT.
            for qq in range(0, NOCT, 2):
                w2ts = []
                for p in ("m1", "m2"):
                    for k in range(4):
                        w2t = wspool.tile([128, 1024], dt.bfloat16, tag="w2t", name="w2t")
                        dma_w2(
                            w2t[:],
                            w2r[p][k * 128:(k + 1) * 128, qq * 512:(qq + 2) * 512],
                        )
                        w2ts.append((w2t, (p, k)))
                for hh in range(2):
                    q = qq + hh
                    P = l2ps.tile([128, 4 * BS], dt.float32, tag="l2p", name="l2p")
                    # one accumulation group at a time per PSUM bank: finish
                    # each j-slice group (8 matmuls + bias) before the next
                    for j in range(4):
                        off = hh * 512 + j * 128
                        for idx, (w2t, pk) in enumerate(w2ts):
                            nc.tensor.matmul(
                                P[:, j * BS:(j + 1) * BS],
                                w2t[:, off: off + 128],
                                h2T[pk][:],
                                start=(idx == 0),
                                stop=(idx == 7) and not nz_b2,
                            )
                        if nz_b2:
                            nc.tensor.matmul(
                                P[:, j * BS:(j + 1) * BS],
                                b2r_sb[0:1, q * 512 + j * 128: q * 512 + (j + 1) * 128],
                                ones1[:],
                                start=False, stop=True,
                            )
                    # UT[0:64, q*2048 : +1024] = P[0:64]; next 1024 = P[64:128]
                    half = 4 * BS                 # 1024 rows of UT per P-half
                    base = q * 2 * half
                    if q % 2 == 0:
                        nc.scalar.activation(
                            UT[0:DH, base:base + half], P[0:64, :], AF.Copy)
                        nc.vector.tensor_copy(
                            UT[0:DH, base + half:base + 2 * half], P[64:128, :])
                    else:
                        nc.vector.tensor_copy(
                            UT[0:DH, base:base + half], P[0:64, :])
                        nc.scalar.activation(
                            UT[0:DH, base + half:base + 2 * half], P[64:128, :], AF.Copy)

            load_dimwise_consts()
            load_xrow()

            l2ps.release()
            madeps.release()
            pspool = tc.alloc_tile_pool(name="ps", bufs=2, space="PSUM")
            ps1 = tc.alloc_tile_pool(name="ps1", bufs=4, space="PSUM")

            # ---------------- dimwise MLP over row blocks ----------------
            # L0 for block b+1 is emitted between L1(b) and final(b): PE fills
            # the stall where it would wait on ACT/DVE producing z2/z2dn(b),
            # and z1(b+1) is ready before L1(b+1) begins.
            def do_L0(b):
                base = b * RBLK
                z1 = {}
                z1d = {}
                for m in range(4):
                    zt = zpool.tile([128, RBLK], dt.bfloat16, tag="z1", name="z1")
                    ps = pspool.tile([128, RBLK], dt.float32, tag="ps", name="psL0")
                    for s in range(2):
                        nc.tensor.matmul(
                            ps[:, s * 512:(s + 1) * 512],
                            dw0x_sb[:, m * 128:(m + 1) * 128],
                            UT[:, base + s * 512: base + (s + 1) * 512],
                            start=True, stop=True,
                        )
                    nc.scalar.activation(
                        zt[:], ps[:], AF.Tanh, bias=bias0c_sb[:, m:m + 1]
                    )
                    sq = sqpool.tile([128, RBLK], dt.bfloat16, tag="sq1", name="sq1")
                    # z1^2 on the (otherwise idle) gpsimd engine: this pass has
                    # a full block of slack (L0(b+1) runs during block b)
                    nc.gpsimd.tensor_tensor(sq[:], zt[:], zt[:], op=OP.mult)
                    zd = z1dpool.tile([128, RBLK], dt.bfloat16, tag="z1d", name="z1d")
                    nc.vector.tensor_scalar(
                        zd[:], sq[:], negac_sb[:, m:m + 1], ac_sb[:, m:m + 1],
                        op0=OP.mult, op1=OP.add,
                    )
                    z1[m] = zt
                    z1d[m] = zd
                return z1, z1d

            # L1 is row-major: per 128-row chunk, z1/z1d chunks are the
            # stationary operands and W1 rows stream; p2T/p2jT land as
            # [128 rows, 512 hid] PSUM tiles. y/jac are then per-partition
            # free-dim reductions fused into DVE passes (accum_out), with
            # d_W2 folded into dw1j (jac) / w2rep (y). Column col = global
            # row-chunk index = i (rows r = i*128 + b), so y_sb is [b, i].
            cur = do_L0(0)
            for b in range(NBLK):
                if b == 8:
                    nc.sync.dma_start(out=y_out[:, 0:64], in_=y_sb[:, 0:64])
                    nc.gpsimd.dma_start(out=j_out[:, 0:64], in_=j_sb[:, 0:64])
                z1, z1d = cur
                if b + 1 < NBLK:
                    cur = do_L0(b + 1)
                for c in range(8):
                    col = b * 8 + c
                    csl = slice(c * 128, (c + 1) * 128)
                    p2 = ps1.tile([128, H], dt.float32, tag="ps1", name="p2ps")
                    p2j = ps1.tile([128, H], dt.float32, tag="ps1", name="p2jps")
                    for k in range(4):
                        nc.tensor.matmul(
                            p2[:], z1[k][:, csl], dw1_sb[:, k * H:(k + 1) * H],
                            start=(k == 0), stop=(k == 3) and not nz_b1,
                        )
                    if nz_b1:
                        nc.tensor.matmul(
                            p2[:], ones1[0:1, 0:128], b1row_sb[:],
                            start=False, stop=True,
                        )
                    for k in range(4):
                        nc.tensor.matmul(
                            p2j[:], z1d[k][:, csl], dw1j_sb[:, k * H:(k + 1) * H],
                            start=(k == 0), stop=(k == 3),
                        )
                    z2t = z2pool.tile([128, H], dt.bfloat16, tag="z2", name="z2")
                    nc.scalar.activation(z2t[:], p2[:], AF.Tanh)
                    sq = sqpool.tile([128, H], dt.bfloat16, tag="sq2", name="sq2")
                    nc.gpsimd.tensor_tensor(sq[:], z2t[:], z2t[:], op=OP.mult)
                    yscr = z2dpool.tile([128, H], dt.bfloat16, tag="yscr", name="yscr")
                    # (z2 * 1.0) * w2rep summed -> y  (ttr is broken on HW)
                    nc.vector.scalar_tensor_tensor(
                        yscr[:], z2t[:], 1.0, w2rep_sb[:],
                        op0=OP.mult, op1=OP.mult,
                        accum_out=y_sb[:, col:col + 1],
                    )
                    jscr = z2dpool.tile([128, H], dt.bfloat16, tag="jscr", name="jscr")
                    # (sq2 - 1) * p2j = (1 - z2^2) * (w2 . p2d) summed -> jac
                    nc.vector.scalar_tensor_tensor(
                        jscr[:], sq[:], 1.0, p2j[:],
                        op0=OP.subtract, op1=OP.mult,
                        accum_out=j_sb[:, col:col + 1],
                    )

            nc.sync.dma_start(out=y_out[:, 64:], in_=y_sb[:, 64:])
            nc.gpsimd.dma_start(out=j_out[:, 64:], in_=j_sb[:, 64:])
            ps1.release()
            pspool.release()
        for p in reversed(_pools):
            p.release()

    nc.compile()
    _NC_CACHE[key] = nc
    return nc


def _host_prep(inputs):
    """Build the per-core input maps (numpy only)."""
    key = tuple(sorted((k, id(v)) for k, v in inputs.items()))
    if key in _PREP_CACHE:
        return _PREP_CACHE[key]
    t = np.asarray(inputs["t"], np.float32)
    x = np.asarray(inputs["x"], np.float32)

    common = {}
    w2rs = {}
    for p in ("m1", "m2"):
        M = _masks(p)
        common[f"w0m_{p}"] = _bf(np.asarray(inputs[f"{p}_W0"], np.float32) * M[0])
        common[f"w1m_{p}"] = _bf(np.asarray(inputs[f"{p}_W1"], np.float32) * M[1])
        w2rs[p] = _w2r_both(np.asarray(inputs[f"{p}_W2"], np.float32), M[3])
        common[f"b0c_{p}"] = _chunk_col(inputs[f"{p}_b0"])
        common[f"b1c_{p}"] = _chunk_col(inputs[f"{p}_b1"])
    b2s = np.asarray(inputs["m1_b2"], np.float32) + np.asarray(inputs["m2_b2"], np.float32)
    ds_maps = []
    for ds in range(ND):
        dm = {f"w2r_{p}": w2rs[p][ds] for p in ("m1", "m2")}
        if np.any(b2s != 0):
            dm["b2r"] = _bf(b2s[_l2_perm(ds * DCORE)].reshape(1, -1))
        ds_maps.append(dm)

    d_W0 = np.asarray(inputs["d_W0"], np.float32)
    d_b0 = np.asarray(inputs["d_b0"], np.float32)
    # u feature order on device: [h0..h63 (parts 0..63), x (part 64)]
    common["dw0x"] = _bf(np.concatenate([d_W0[2:, :], d_W0[1:2, :]], axis=0))
    common["bias0c"] = _chunk_col(d_b0 + t[0] * d_W0[0, :])
    a = d_W0[1, :]
    common["ac"] = _chunk_col(a)
    common["negac"] = _chunk_col(-a)
    dW1 = np.asarray(inputs["d_W1"], np.float32)
    common["dw1"] = _bf(dW1)
    w2 = np.asarray(inputs["d_W2"], np.float32)[:, 0]
    common["dw1j"] = _bf(dW1 * (-w2)[None, :])
    common["w2rep"] = _bf(np.tile(w2[None, :], (128, 1)))
    d_b1 = np.asarray(inputs["d_b1"], np.float32)
    nz_b1 = bool(np.any(d_b1 != 0))
    if nz_b1:
        common["b1row"] = _bf(d_b1.reshape(1, H))

    in_maps = []
    for c in range(NCORES):
        bs, ds = c % NB, c // NB
        xs = x[bs * BS:(bs + 1) * BS, :]
        m = dict(common)
        m.update(ds_maps[ds])
        m["xT"] = _bf(xs.T)
        m["xflat"] = _bf(
            np.ascontiguousarray(xs[:, ds * DCORE:(ds + 1) * DCORE].T).reshape(1, ROWS))
        in_maps.append(m)
    _PREP_CACHE[key] = in_maps
    return in_maps


def kernel(**inputs):
    from concourse.bass_utils import run_bass_kernel_spmd

    nz_b1 = bool(np.any(np.asarray(inputs["d_b1"], np.float32) != 0))
    b2s = np.asarray(inputs["m1_b2"], np.float32) + np.asarray(inputs["m2_b2"], np.float32)
    nz_b2 = bool(np.any(b2s != 0))
    nc = _build_nc(nz_b1, nz_b2)
    in_maps = _host_prep(inputs)
    res = run_bass_kernel_spmd(nc, in_maps, list(range(NCORES)))

    d_b2 = np.asarray(inputs["d_b2"], np.float32)
    y = np.empty((B, D), np.float32)
    jac = np.empty((B, D), np.float32)
    for c in range(NCORES):
        bs, ds = c % NB, c // NB
        rs, cs = slice(bs * BS, (bs + 1) * BS), slice(ds * DCORE, (ds + 1) * DCORE)
        y[rs, cs] = _unmat(res.results[c]["y_mat"]) + d_b2[0]
        jac[rs, cs] = _unmat(res.results[c]["jac_mat"])
    return y, jac


def _unmat(m):
    """[128, 128] device mat (p, col) -> [BS, DCORE]: row (col%2)*128+p, dim col//2."""
    return np.asarray(m, np.float32).reshape(128, DCORE, 2).transpose(2, 0, 1).reshape(BS, DCORE)
